# revision 1
# baseline (speedup 1.0000x reference)
"""Tensor-parallel causal attention layer (RoPE) for 8 Trainium2 NeuronCores.

Problem: nn_AttentionTier (B=4, T=2048, D=1024, H=16, Dh=64), fp32 I/O.

Sharding: DP=4 over batch x TP=2 over heads (8 heads per core).
  core c -> batch c//2, head group c%2 (heads 8*(c%2) .. 8*(c%2)+8).

v4 — single software-pipelined region: the QKV projection of token block
tb+1 is issued right after attention of block tb, so projection matmuls
(PE) fill the gaps while attention is ACT(exp)-bound, and the PE never
idles long enough for the HAM clock-throttle to re-engage.

Per core:
  - bf16 everywhere on-chip except PSUM accumulation (fp32) and the rope
    sin/cos combine inputs; x / W_qkv / W_v loaded as bf16.
  - QK projection in transposed layout; RoPE via rotation-permutation
    matmul + cos/sin combines split across ACT/DVE/GPSIMD.
  - Attention: per k-chunk the two heads of a plane get score matmuls on
    disjoint PE row groups (tile_position (0,0)/(64,0), concurrent); one
    exp per chunk covers both heads; ones-augmented V gives row sums.
  - Softmax: rowsums gathered to partitions {0,32,64,96}, one ln+exp(-x)
    pass, reciprocal broadcast via K=1 matmuls on packed row groups.
  - Out-projection partials sharded by TOKEN half; pairwise bf16
    ReduceScatter leaves each core the full-feature output for its
    256-token half; stored bf16 and upcast to fp32 on the host.
  - PSUM: score pairs 2x2 banks, one 2-bank o-accumulator pair, one
    2-slot scratch pool shared by projection/rot/V/out-proj/broadcast.
"""

import sys

sys.path.insert(0, "/opt/trn_rl_repo")

import numpy as np

B, T, D = 4, 2048, 1024
H, Dh = 16, 64
N_CORES = 8
P = 128
TB = 512          # token block (matmul moving dim)
HTB = TB // 2     # per-core token half after the exchange
NTB = T // TB     # 4
NCC = D // P      # 8 contraction chunks
HLOC = H // 2     # heads per core

_CACHE = {}


def _patch_act_tables():
    """Force every ACT function we use into one table set so bacc emits a
    single hoisted InstLoadActFuncSet instead of thrashing between the
    exp- and ln-anchored sets on every softmax row."""
    import functools
    import concourse.mybir as mybir
    from concourse import bacc, hw_specs

    if getattr(bacc.get_activation_tables, "_attn_patched", False):
        return
    orig = hw_specs.get_activation_tables
    AF = mybir.ActivationFunctionType
    ours = {AF.Exp, AF.Ln, AF.Copy, AF.Identity}

    @functools.cache
    def patched(module_arch):
        tabs = dict(orig(module_arch))
        return {
            name: (fns if name == "natural_log_exp_and_others"
                   else set(fns) - ours)
            for name, fns in tabs.items()
        }

    patched._attn_patched = True
    bacc.get_activation_tables = patched


def _build_program(reps=1):
    import concourse.bass as bass  # noqa: F401
    import concourse.mybir as mybir
    import concourse.tile as tile
    from concourse import bacc

    _patch_act_tables()

    f32 = mybir.dt.float32
    bf16 = mybir.dt.bfloat16
    AF = mybir.ActivationFunctionType

    nc = bacc.Bacc("TRN2", target_bir_lowering=False, debug=False,
                   num_devices=N_CORES)

    # ---- DRAM I/O ----
    xT_d = nc.dram_tensor("xT", [D, T], bf16, kind="ExternalInput").ap()
    wqkT_d = nc.dram_tensor("wqkT", [D, D], bf16, kind="ExternalInput").ap()
    wvT_d = nc.dram_tensor("wvT", [D, D // 2], bf16,
                           kind="ExternalInput").ap()
    woutT_d = nc.dram_tensor("woutT", [D // 2, D], bf16,
                             kind="ExternalInput").ap()
    r2T_d = nc.dram_tensor("r2T", [P, P], bf16, kind="ExternalInput").ap()
    cos2_d = nc.dram_tensor("cos2", [P, T], bf16, kind="ExternalInput").ap()
    sin2_d = nc.dram_tensor("sin2", [P, T], bf16, kind="ExternalInput").ap()
    tri_d = nc.dram_tensor("tri", [P, P], bf16, kind="ExternalInput").ap()
    out_d = nc.dram_tensor("out", [D // 2, T], bf16,
                           kind="ExternalOutput").ap()

    groups = [[0, 1], [2, 3], [4, 5], [6, 7]]

    with tile.TileContext(nc) as tc:
        with tc.tile_pool(name="const", bufs=1) as constp, \
             tc.tile_pool(name="big", bufs=1) as bigp, \
             tc.tile_pool(name="w1", bufs=1) as w1p, \
             tc.tile_pool(name="ph1", bufs=3) as ph1, \
             tc.tile_pool(name="xtp", bufs=2) as xtp, \
             tc.tile_pool(name="att", bufs=6) as attp, \
             tc.tile_pool(name="msc", bufs=2) as mscp, \
             tc.tile_pool(name="aop", bufs=2) as aop, \
             tc.tile_pool(name="dram", bufs=2, space="DRAM") as dramp, \
             tc.tile_pool(name="ps_s", bufs=2, space="PSUM") as ps_s, \
             tc.tile_pool(name="ps_o", bufs=1, space="PSUM") as ps_o, \
             tc.tile_pool(name="ps_x", bufs=2, space="PSUM") as ps_x:

            r2T = constp.tile([P, P], bf16)
            tri = constp.tile([P, P], bf16)
            ones_b = constp.tile([P, P], bf16)
            nc.vector.memset(ones_b[:], 1.0)

            # persistent big tensors (bf16)
            qk = bigp.tile([P, NCC, T], bf16)              # rope'd q^T,k^T
            vbar = bigp.tile([P, T // P, HLOC, Dh + 1], bf16)
            nc.vector.tensor_copy(
                vbar[:, :, :, Dh:Dh + 1],
                ones_b[:, None, :HLOC, None].to_broadcast(
                    [P, T // P, HLOC, 1]))

            wqkT = w1p.tile([P, NCC, D], bf16)
            wvT = w1p.tile([P, NCC, D // 2], bf16)
            woutT = w1p.tile([P, NCC // 2, D], bf16)

            def load_xT(tb):
                t = xtp.tile([P, NCC, TB], bf16, tag="xT")
                for cc in range(NCC):
                    nc.sync.dma_start(
                        t[:, cc],
                        xT_d[cc * P:(cc + 1) * P, tb * TB:(tb + 1) * TB])
                return t

            # first-need DMAs first
            nc.sync.dma_start(wqkT[:, 0], wqkT_d[0:P, :])
            xT0 = load_xT(0)
            nc.sync.dma_start(r2T[:], r2T_d[:])
            nc.sync.dma_start(tri[:], tri_d[:])
            for cc in range(1, NCC):
                nc.sync.dma_start(wqkT[:, cc], wqkT_d[cc * P:(cc + 1) * P, :])
            for cc in range(NCC):
                nc.sync.dma_start(wvT[:, cc], wvT_d[cc * P:(cc + 1) * P, :])

            # ---- emission helpers ----
            cstiles = {}

            def cs_tiles(tb):
                if tb not in cstiles:
                    tsl = slice(tb * TB, (tb + 1) * TB)
                    cosb = ph1.tile([P, TB], bf16, tag="cosb",
                                    name=f"cosb_{tb}")
                    sinb = ph1.tile([P, TB], bf16, tag="sinb",
                                    name=f"sinb_{tb}")
                    nc.sync.dma_start(cosb[:], cos2_d[:, tsl])
                    nc.sync.dma_start(sinb[:], sin2_d[:, tsl])
                    cstiles[tb] = (cosb, sinb)
                return cstiles[tb]

            def proj_qk(tb, xT, oc):
                """one 128-feature chunk of QK projection + rope"""
                tsl = slice(tb * TB, (tb + 1) * TB)
                cosb, sinb = cs_tiles(tb)
                qk_ps = ps_x.tile([P, TB], f32, tag="xps",
                                  name=f"qkps_{tb}_{oc}")
                for cc in range(NCC):
                    nc.tensor.matmul(
                        qk_ps[:], wqkT[:, cc, oc * P:(oc + 1) * P],
                        xT[:, cc, :],
                        start=(cc == 0), stop=(cc == NCC - 1))
                raw = ph1.tile([P, TB], bf16, tag="raw",
                               name=f"raw_{tb}_{oc}")
                nc.scalar.copy(raw[:], qk_ps[:])
                rot_ps = ps_x.tile([P, TB], f32, tag="xps",
                                   name=f"rotps_{tb}_{oc}")
                nc.tensor.matmul(rot_ps[:], r2T[:], raw[:],
                                 start=True, stop=True)
                ta = ph1.tile([P, TB], bf16, tag="ta", name=f"ta_{tb}_{oc}")
                nc.vector.tensor_tensor(
                    ta[:], rot_ps[:], sinb[:], mybir.AluOpType.mult)
                tb_ = ph1.tile([P, TB], bf16, tag="tb_",
                               name=f"tb__{tb}_{oc}")
                nc.gpsimd.tensor_tensor(
                    tb_[:], raw[:], cosb[:], mybir.AluOpType.mult)
                nc.vector.tensor_tensor(
                    qk[:, oc, tsl], ta[:], tb_[:], mybir.AluOpType.add)

            def proj_v(tb, xT, ts):
                """one 128-token chunk of the V projection"""
                v_ps = ps_x.tile([P, D // 2], f32, tag="xps",
                                 name=f"vps_{tb}_{ts}")
                for cc in range(NCC):
                    nc.tensor.matmul(
                        v_ps[:], xT[:, cc, ts * P:(ts + 1) * P],
                        wvT[:, cc, :],
                        start=(cc == 0), stop=(cc == NCC - 1))
                tc_idx = tb * (TB // P) + ts
                nc.scalar.copy(
                    vbar[:, tc_idx, :, 0:Dh],
                    v_ps[:].rearrange("p (h d) -> p h d", h=HLOC))

            def proj_slice(tb, xT, hp):
                """interleaved projection slice: the planes head-pair hp of
                block tb's attention needs first, plus one V chunk"""
                proj_qk(tb, xT, hp)
                proj_qk(tb, xT, NCC // 2 + hp)
                proj_v(tb, xT, hp)

            qstate = {}

            def attn_state(qb):
                if qb not in qstate:
                    sstack = mscp.tile([P, 2, TB], bf16, tag="sstack",
                                       name=f"sstack_{qb}")
                    rstack = mscp.tile([P, 2, TB], bf16, tag="rstack",
                                       name=f"rstack_{qb}")
                    qstate[qb] = (sstack, rstack, [None] * (HLOC // 2))
                return qstate[qb]

            def attn_hp(qb, hp, filler=None):
                """attention for head pair hp of q block qb + rowsum gather.
                `filler` is a no-arg callable emitting one pending projection
                chain; invoked between chunks so the PE static order always
                holds independent work inside exp-latency windows."""
                fill = filler if filler is not None else (lambda: None)
                sstack, rstack, osbs = attn_state(qb)
                qsl = slice(qb * TB, (qb + 1) * TB)
                h1, h2 = 2 * hp, 2 * hp + 1
                kpl = NCC // 2 + hp
                qpl = hp

                def smm(s_pair, kc, fsl):
                    ks = slice(kc * P, (kc + 1) * P)
                    nc.tensor.matmul(
                        s_pair[:, 0, fsl],
                        qk[0:Dh, kpl, ks], qk[0:Dh, qpl, qsl][:, fsl],
                        start=True, stop=True, tile_position=(0, 0))
                    nc.tensor.matmul(
                        s_pair[:, 1, fsl],
                        qk[Dh:P, kpl, ks], qk[Dh:P, qpl, qsl][:, fsl],
                        start=True, stop=True, tile_position=(64, 0))

                o_pair = ps_o.tile([Dh + 1, 2, TB], f32, tag="ops",
                                   name=f"op_{qb}_{hp}")
                for kc in range(4 * qb):
                    s_pair = ps_s.tile([P, 2, TB], f32, tag="sps",
                                       name=f"sp_{qb}_{hp}_{kc}")
                    smm(s_pair, kc, slice(0, TB))
                    pt = attp.tile([P, 2, TB], bf16, tag="pt",
                                   name=f"pt_{qb}_{hp}_{kc}")
                    nc.scalar.activation(
                        pt[:], s_pair[:], AF.Exp, scale=0.125)
                    for j, h in ((0, h1), (1, h2)):
                        nc.tensor.matmul(
                            o_pair[:, j, :], vbar[:, kc, h, :], pt[:, j, :],
                            start=(kc == 0), stop=False,
                            skip_group_check=True)
                    fill()
                for cr in range(4):
                    kc = 4 * qb + cr
                    qo = cr * P
                    fsl = slice(qo, TB)
                    s_pair = ps_s.tile([P, 2, TB], f32, tag="sps",
                                       name=f"spd_{qb}_{hp}_{cr}")
                    smm(s_pair, kc, fsl)
                    pt = attp.tile([P, 2, TB], bf16, tag="pt",
                                   name=f"ptd_{qb}_{hp}_{cr}")
                    nc.scalar.activation(
                        pt[:, :, fsl], s_pair[:, :, fsl], AF.Exp,
                        scale=0.125)
                    nc.vector.tensor_tensor(
                        pt[:, :, qo:qo + P], pt[:, :, qo:qo + P],
                        tri[:, None, :].to_broadcast([P, 2, P]),
                        mybir.AluOpType.mult)
                    for j, h in ((0, h1), (1, h2)):
                        nc.tensor.matmul(
                            o_pair[:, j, fsl], vbar[:, kc, h, :],
                            pt[:, j, fsl],
                            start=(kc == 0), stop=(cr == 3),
                            skip_group_check=True)
                    fill()

                # evacuate PSUM; one DMA gathers both heads' rowsum rows to
                # partition 32*hp of sstack
                osb = mscp.tile([Dh + 1, 2, TB], bf16, tag=f"osb{hp % 2}",
                                name=f"osb_{qb}_{hp}")
                nc.vector.tensor_copy(osb[:], o_pair[:])
                nc.gpsimd.dma_start(
                    sstack[32 * hp:32 * hp + 1, :, :], osb[Dh:Dh + 1, :, :])
                osbs[hp] = osb

            def norm_recip(qb):
                """batched reciprocal of all 8 rowsums: 1/x = exp(-ln x)"""
                sstack, rstack, _ = attn_state(qb)
                nc.scalar.activation(sstack[:], sstack[:], AF.Ln)
                nc.scalar.activation(rstack[:], sstack[:], AF.Exp,
                                     scale=-1.0)

            def norm_apply(qb, aout):
                _, rstack, osbs = attn_state(qb)
                for h in range(HLOC):
                    hp, j = h // 2, h % 2
                    rp = 32 * hp
                    b_ps = ps_x.tile([Dh, TB], f32, tag="xps",
                                     name=f"bps_{qb}_{h}")
                    nc.tensor.matmul(
                        b_ps[:], ones_b[rp:rp + 1, 0:Dh],
                        rstack[rp:rp + 1, j, :],
                        start=True, stop=True, tile_position=(rp, 0))
                    nc.vector.tensor_tensor(
                        aout[64 * j:64 * j + Dh, hp, :],
                        osbs[hp][0:Dh, j, :], b_ps[:],
                        mybir.AluOpType.mult)

            def outproj(qb, aout):
                """out-projection partials split into two feature-half
                groups, each with its own pairwise feature-sharded RS so
                consecutive half-RSs pipeline on the collective stream and
                the final exposed RS is half-size. Core at pair-rank r ends
                with global output features r*512..r*512+512 (local rows
                0:512 of out_d) for all tokens."""
                for g, ecs in ((0, (0, 1, 4, 5)), (1, (2, 3, 6, 7))):
                    part = dramp.tile([D // 2, TB], bf16, tag="part",
                                      name=f"part_{qb}_{g}")
                    for i, ec in enumerate(ecs):
                        f_ps = ps_x.tile([P, TB], f32, tag="xps",
                                         name=f"fps_{qb}_{g}_{ec}")
                        for cc in range(NCC // 2):
                            nc.tensor.matmul(
                                f_ps[:], woutT[:, cc, ec * P:(ec + 1) * P],
                                aout[:, cc, :],
                                start=(cc == 0), stop=(cc == NCC // 2 - 1))
                        fsb = mscp.tile([P, TB], bf16, tag="fsb",
                                        name=f"fsb_{qb}_{g}_{ec}")
                        nc.vector.tensor_copy(fsb[:], f_ps[:])
                        # shard s = (ec >= 4): features for pair-rank s
                        row = ((0 if ec < 4 else 256)
                               + (ec - (0 if ec < 4 else 4) - 2 * g) * P)
                        nc.sync.dma_start(part[row:row + P, :], fsb[:])
                    rs_o = dramp.tile([D // 4, TB], bf16, tag="rs_o",
                                      name=f"rso_{qb}_{g}")
                    nc.gpsimd.collective_compute(
                        "ReduceScatter", mybir.AluOpType.add,
                        replica_groups=groups,
                        ins=[part.opt()], outs=[rs_o.opt()])
                    nc.sync.dma_start(
                        out_d[g * (D // 4):(g + 1) * (D // 4),
                              qb * TB:(qb + 1) * TB], rs_o[:])

            # ---- software-pipelined emission ----
            # projection chains of block tb+1 are drip-fed between attention
            # chunks of block tb (chunk-granular), so ACT keeps an unbroken
            # exp stream while the PE interleaves both; the next block's
            # first head pair sits between the reciprocal and the broadcast
            # ladder to cover the rowsum-gather latency.
            xts = {0: xT0}
            for oc in range(NCC):
                proj_qk(0, xts[0], oc)
            for ts in range(TB // P):
                proj_v(0, xts[0], ts)
            xts[1] = load_xT(1)

            # woutT queued after xT(1) — not needed until the first out-proj
            nc.sync.dma_start(
                woutT[:], woutT_d.rearrange("(cc p) o -> p cc o", p=P))

            attn_hp(0, 0)
            for qb in range(NTB):
                if qb + 1 < NTB:
                    ntb = qb + 1
                    xT = xts[ntb]
                    work = [lambda oc=oc: proj_qk(ntb, xT, oc)
                            for oc in (0, NCC // 2)]
                    work += [lambda ts=ts: proj_v(ntb, xT, ts)
                             for ts in range(TB // P)]
                    work += [lambda oc=oc: proj_qk(ntb, xT, oc)
                             for hp in range(1, HLOC // 2)
                             for oc in (hp, NCC // 2 + hp)]
                    nchunks = 3 * (4 * qb + 4)      # chunks in hps 1..3
                    stride = max(1, nchunks // len(work))
                    state = {"i": 0, "n": 0}

                    def filler():
                        state["n"] += 1
                        if state["n"] % stride == 0 and state["i"] < len(work):
                            work[state["i"]]()
                            state["i"] += 1
                    if qb + 2 < NTB:
                        xts[qb + 2] = load_xT(qb + 2)
                else:
                    work, filler = [], None
                for hp in range(1, HLOC // 2):
                    attn_hp(qb, hp, filler)
                # flush any proj chains the stride arithmetic left over
                if filler is not None:
                    while state["i"] < len(work):
                        work[state["i"]]()
                        state["i"] += 1
                norm_recip(qb)
                if qb + 1 < NTB:
                    attn_hp(qb + 1, 0)
                aout = aop.tile([P, NCC // 2, TB], bf16, tag="aout",
                                name=f"aout_{qb}")
                norm_apply(qb, aout)
                outproj(qb, aout)

    nc.compile()
    return nc


def _host_inputs(x, W_qkv, W_out):
    """Per-core input dicts."""
    import ml_dtypes
    bf = ml_dtypes.bfloat16
    x = np.ascontiguousarray(np.asarray(x, dtype=np.float32))
    W_qkv = np.asarray(W_qkv, dtype=np.float32)
    W_out = np.asarray(W_out, dtype=np.float32)

    # rope tables, transposed layout, 2-head stack
    inv = 1.0 / (10000.0 ** (np.arange(0, Dh, 2, dtype=np.float64) / Dh))
    ang = np.outer(np.arange(T, dtype=np.float64), inv)        # (T, 32)
    emb = np.concatenate([ang, ang], axis=1)                   # (T, 64)
    cosT = np.cos(emb).astype(np.float32).T                    # (64, T)
    sinT = np.sin(emb).astype(np.float32).T
    cos2 = np.ascontiguousarray(
        np.concatenate([cosT, cosT], 0)).astype(bf)            # (128, T)
    sin2 = np.ascontiguousarray(
        np.concatenate([sinT, sinT], 0)).astype(bf)

    # rotation matrix: rot(q) = R @ q ; lhsT = R2.T
    R = np.zeros((Dh, Dh), np.float32)
    for d in range(Dh // 2):
        R[d, d + Dh // 2] = -1.0
        R[d + Dh // 2, d] = 1.0
    R2 = np.zeros((P, P), np.float32)
    R2[:Dh, :Dh] = R
    R2[Dh:, Dh:] = R
    r2T = np.ascontiguousarray(R2.T).astype(bf)

    # triangular mask in scores^T layout: keep k <= q
    tri = np.triu(np.ones((P, P), np.float32)).astype(bf)

    ins = []
    for c in range(N_CORES):
        b, tp = c // 2, c % 2
        heads = range(8 * tp, 8 * tp + 8)
        wq = np.concatenate([W_qkv[64 * h: 64 * h + 64] for h in heads], 0)
        wk = np.concatenate(
            [W_qkv[D + 64 * h: D + 64 * h + 64] for h in heads], 0)
        wv = np.concatenate(
            [W_qkv[2 * D + 64 * h: 2 * D + 64 * h + 64] for h in heads], 0)
        wqkT = np.ascontiguousarray(
            np.concatenate([wq, wk], 0).T).astype(bf)               # (1024,1024)
        wvT = np.ascontiguousarray(wv.T).astype(bf)                 # (1024,512)
        woutT = np.ascontiguousarray(
            W_out[:, 512 * tp: 512 * tp + 512].T).astype(bf)        # (512,1024)
        ins.append({
            "xT": np.ascontiguousarray(x[b].T).astype(bf),
            "wqkT": wqkT, "wvT": wvT, "woutT": woutT,
            "r2T": r2T, "cos2": cos2, "sin2": sin2, "tri": tri,
        })
    return ins


def kernel(x, W_qkv, W_out):
    import time
    from concourse.bass_utils import run_bass_kernel_spmd

    if "nc" not in _CACHE:
        _CACHE["nc"] = _build_program()
    nc = _CACHE["nc"]
    ins = _host_inputs(x, W_qkv, W_out)
    res = None
    for attempt in range(3):
        try:
            res = run_bass_kernel_spmd(nc, ins, list(range(N_CORES)))
            break
        except Exception:
            # if a previous process' device teardown raced our startup the
            # first execution can die; give the worker time to come back and
            # drop any broken backend handles before retrying
            if attempt == 2:
                raise
            time.sleep(30)
            try:
                import jax
                jax.clear_caches()
                jax.clear_backends()
            except Exception:
                pass
    out = np.empty((B, T, D), dtype=np.float32)
    for c in range(N_CORES):
        b, tp = c // 2, c % 2
        o = np.asarray(res.results[c]["out"]).astype(np.float32)  # (D//2, T)
        out[b, :, 512 * tp: 512 * tp + 512] = o.T
    return out

